# revision 2
# baseline (speedup 1.0000x reference)
"""DSMIL pooling kernel for 8 Trainium2 NeuronCores.

Sharding: B=4 bags x 2-way sequence split of N=16384 -> 8 shards of
[8192, 1024].

Launch 1 (per core): h0^T = (x@we) in fp8e4 DoubleRow (2-4x PE rate,
4x less HBM x-traffic than f32), stored to DRAM as bf16 WITHOUT the
be bias (the bias is a constant shift of the attention scores -> it
cancels in softmax; the instance-logit / p-projection constants are
re-added on host). The same fp8 x tiles also feed a fused rank-4
matmul computing lp = x @ [we@wi | we@wb_bot] (instance logits +
p-projection p_n = h_n@wb_bot). Since attn_bag only enters the output
through attn_bag @ wb_bot, the attention-weighted sum over h [N,512]
reduces to a weighted sum over p [N,2] -- done on host from lp.

Host glue: exact (f64) critical-instance embedding from the argmax of
the device-computed instance scores: crit = x[iw]@we + be, then
q = crit@wq+bq, v = wk@q/sqrt(E).

Launch 2 (per core): s = v^T h0 matvec over the bf16 h0^T (PE), ships
s [NS] back; softmax + weighted p-sum + final head run on host (exact,
tiny). fp8 weights are pre-scaled (we*32, W2*64) on host to dodge the
e4m3 denormal range; the device copies h0 out with scale 1/32 and the
host divides lp by 64.
"""

import numpy as np
import ml_dtypes

import concourse.mybir as mybir
import concourse.tile as tile
from concourse import bacc
from concourse.bass import ts
from concourse.bass_utils import run_bass_kernel_spmd

# ---- tile-tail drain workaround (this walrus build rejects >1 sync-wait
# per instruction on the kernel-tail Drain) ----
from concourse.vector_clock import ScopedClock

_MAX_WAITS = 1


def _patched_drain_and_barrier(self, tick_clock, wait_clock):
    probe = self.nc.sync.nop(nofuse=True, hint="tile_drain_waits")
    wait_clock.add_sem_waits(probe.ins, ScopedClock({None: tick_clock.global_clock}))
    si = probe.ins.sync_info
    waits = list(si.on_wait) if si is not None and si.on_wait else []
    if len(waits) > _MAX_WAITS:
        si.on_wait = waits[:_MAX_WAITS]
        rest = waits[_MAX_WAITS:]
        for k in range(0, len(rest), _MAX_WAITS):
            extra = self.nc.sync.nop(nofuse=True, hint="tile_drain_waits")
            esi = extra.ins.sync_info
            if esi is None:
                extra.ins.sync_info = mybir.SyncInfo(
                    on_wait=rest[k : k + _MAX_WAITS], on_update=[]
                )
            else:
                esi.on_wait = rest[k : k + _MAX_WAITS]
    self.nc.sync.drain()
    self.nc.all_engine_barrier()
    popped = self.nc._tile_sem_poison_stack.pop()
    assert popped is self._sem_poison
    self.nc.clear_and_free_semaphores(list(self.sems.allocated().values()))
    self.nc.all_engine_barrier()


tile.TileContext._drain_and_barrier = _patched_drain_and_barrier

F32 = mybir.dt.float32
BF16 = mybir.dt.bfloat16
F8 = mybir.dt.float8e4
NPF8 = ml_dtypes.float8_e4m3
NPBF16 = ml_dtypes.bfloat16

B, N, D, E, C = 4, 16384, 1024, 512, 2
NCORES = 8
NS = N // 2          # per-core sequence shard
DB = D // 128        # 8
EB = E // 128        # 4
NTD = 2048           # DMA tile width (n)
NTI = 512            # inner compute tile width (psum free dim)
N_DMAT = NS // NTD   # 4
N_INNER = NTD // NTI  # 4

WE_SCALE = 32.0      # pre-scale for fp8 we (dodges e4m3 denormals)
W2_SCALE = 64.0      # pre-scale for fp8 [we@wi | we@wb_bot]

_cache = {}


def _build_launch1(use_dr=True):
    nc = bacc.Bacc(None, target_bir_lowering=False)
    x_d = nc.dram_tensor("xb8", [128, DB, NS], F8, kind="ExternalInput")
    we_d = nc.dram_tensor("we8", [128, DB, E], F8, kind="ExternalInput")
    w2_d = nc.dram_tensor("w28", [128, DB, 4], F8, kind="ExternalInput")
    h_d = nc.dram_tensor("h0T", [128, EB, NS], BF16, kind="ExternalOutput")
    lp_d = nc.dram_tensor("lp", [4, NS], F32, kind="ExternalOutput")

    DRM = mybir.MatmulPerfMode.DoubleRow if use_dr else None

    with tile.TileContext(nc) as tc:
        with (
            tc.tile_pool(name="wpool", bufs=1) as wp,
            tc.tile_pool(name="xpool", bufs=2) as xp,
            tc.tile_pool(name="hpool", bufs=2) as hp,
            tc.tile_pool(name="lpool", bufs=1) as lpp,
            tc.tile_pool(name="psum", bufs=1, space="PSUM") as pp,
        ):
            we_sb = wp.tile([128, DB, E], F8)
            nc.sync.dma_start(we_sb[:], we_d[:])
            w2_sb = wp.tile([128, DB, 4], F8)
            nc.sync.dma_start(w2_sb[:], w2_d[:])
            lp_sb = lpp.tile([4, NS], F32)

            for td in range(N_DMAT):
                x_t = xp.tile([128, DB, NTD], F8, tag="xt", name="x_t")
                nc.sync.dma_start(x_t[:], x_d[:, :, ts(td, NTD)])
                hstg = hp.tile([128, EB, NTD], BF16, tag="hstg", name="hstg")
                for i in range(N_INNER):
                    nt = td * N_INNER + i
                    xs = x_t[:, :, ts(i, NTI)]
                    phs = [
                        pp.tile([128, NTI], F32, tag=f"ph{eb}", name=f"ph{eb}")
                        for eb in range(EB)
                    ]
                    plp = pp.tile([4, NTI], F32, tag="plp", name="plp")
                    if use_dr:
                        for eb in range(EB):
                            for dbp in range(DB // 2):
                                nc.tensor.matmul(
                                    phs[eb][:],
                                    lhsT=we_sb[:, 2 * dbp : 2 * dbp + 2, ts(eb, 128)],
                                    rhs=x_t[:, 2 * dbp : 2 * dbp + 2, ts(i, NTI)],
                                    start=(dbp == 0),
                                    stop=(dbp == DB // 2 - 1),
                                    perf_mode=DRM,
                                )
                        for dbp in range(DB // 2):
                            nc.tensor.matmul(
                                plp[:],
                                lhsT=w2_sb[:, 2 * dbp : 2 * dbp + 2, 0:4],
                                rhs=x_t[:, 2 * dbp : 2 * dbp + 2, ts(i, NTI)],
                                start=(dbp == 0),
                                stop=(dbp == DB // 2 - 1),
                                perf_mode=DRM,
                            )
                    else:
                        for eb in range(EB):
                            for db in range(DB):
                                nc.tensor.matmul(
                                    phs[eb][:],
                                    lhsT=we_sb[:, db, ts(eb, 128)],
                                    rhs=xs[:, db, :],
                                    start=(db == 0),
                                    stop=(db == DB - 1),
                                )
                        for db in range(DB):
                            nc.tensor.matmul(
                                plp[:],
                                lhsT=w2_sb[:, db, 0:4],
                                rhs=xs[:, db, :],
                                start=(db == 0),
                                stop=(db == DB - 1),
                            )
                    # PSUM -> SBUF with 1/WE_SCALE, cast to bf16; split engines
                    for eb in range(EB):
                        dst = hstg[:, eb, ts(i, NTI)]
                        if eb % 2 == 0:
                            nc.vector.tensor_scalar_mul(dst, phs[eb][:], 1.0 / WE_SCALE)
                        else:
                            nc.scalar.activation(
                                dst, phs[eb][:],
                                mybir.ActivationFunctionType.Copy,
                                scale=1.0 / WE_SCALE,
                            )
                    nc.vector.tensor_copy(lp_sb[:, ts(nt, NTI)], plp[:])
                nc.sync.dma_start(h_d[:, :, ts(td, NTD)], hstg[:])
            nc.sync.dma_start(lp_d[:], lp_sb[:])
    nc.compile()
    return nc


def _build_launch2():
    nc = bacc.Bacc(None, target_bir_lowering=False)
    h_d = nc.dram_tensor("h0T", [128, EB, NS], BF16, kind="ExternalInput")
    v_d = nc.dram_tensor("vcol", [128, EB], BF16, kind="ExternalInput")
    s_d = nc.dram_tensor("s", [1, NS], F32, kind="ExternalOutput")

    with tile.TileContext(nc) as tc:
        with (
            tc.tile_pool(name="wpool", bufs=1) as wp,
            tc.tile_pool(name="hpool", bufs=2) as hp,
            tc.tile_pool(name="spool", bufs=1) as sp,
            tc.tile_pool(name="psum", bufs=1, space="PSUM") as pp,
        ):
            v_sb = wp.tile([128, EB], BF16)
            nc.sync.dma_start(v_sb[:], v_d[:])
            s_sb = sp.tile([1, NS], F32)
            for td in range(N_DMAT):
                h_t = hp.tile([128, EB, NTD], BF16, tag="ht", name="h_t")
                nc.sync.dma_start(h_t[:], h_d[:, :, ts(td, NTD)])
                for i in range(N_INNER):
                    nt = td * N_INNER + i
                    ps = pp.tile([1, NTI], F32, tag=f"ps{nt % 2}", name="ps")
                    for eb in range(EB):
                        nc.tensor.matmul(
                            ps[:],
                            lhsT=v_sb[:, eb : eb + 1],
                            rhs=h_t[:, eb, ts(i, NTI)],
                            start=(eb == 0),
                            stop=(eb == EB - 1),
                        )
                    if nt % 2 == 0:
                        nc.vector.tensor_copy(s_sb[0:1, ts(nt, NTI)], ps[:])
                    else:
                        nc.scalar.activation(
                            s_sb[0:1, ts(nt, NTI)], ps[:],
                            mybir.ActivationFunctionType.Copy,
                        )
            nc.sync.dma_start(s_d[:], s_sb[:])
    nc.compile()
    return nc


def _get_launches():
    if "l1" not in _cache:
        try:
            _cache["l1"] = _build_launch1(use_dr=True)
        except Exception:
            _cache["l1"] = _build_launch1(use_dr=False)
    if "l2" not in _cache:
        _cache["l2"] = _build_launch2()
    return _cache["l1"], _cache["l2"]


def kernel(x, we, be, wi, bi, wq, bq, wk, bk, wb, bb):
    x = np.asarray(x, dtype=np.float32)
    we = np.asarray(we, dtype=np.float32)
    be = np.asarray(be, dtype=np.float32)
    wi = np.asarray(wi, dtype=np.float32)
    bi = np.asarray(bi, dtype=np.float32)
    wq = np.asarray(wq, dtype=np.float32)
    bq = np.asarray(bq, dtype=np.float32)
    wk = np.asarray(wk, dtype=np.float32)
    bk = np.asarray(bk, dtype=np.float32)
    wb = np.asarray(wb, dtype=np.float32)
    bb = np.asarray(bb, dtype=np.float32)

    l1, l2 = _get_launches()

    # host-packed fp8 operands
    we8 = np.ascontiguousarray(
        (we * WE_SCALE).astype(NPF8).reshape(DB, 128, E).transpose(1, 0, 2)
    )
    W2 = np.concatenate([we @ wi, we @ wb[E:]], axis=1)  # [D, 4]
    w28 = np.ascontiguousarray(
        (W2 * W2_SCALE).astype(NPF8).reshape(DB, 128, 4).transpose(1, 0, 2)
    )

    in_maps1 = []
    for c in range(NCORES):
        b, half = divmod(c, 2)
        xs = x[b, half * NS : (half + 1) * NS, :]            # [NS, D] f32
        x8 = xs.astype(NPF8)                                  # [NS, D] fp8
        xb8 = np.ascontiguousarray(
            x8.reshape(NS, DB, 128).transpose(2, 1, 0)        # [128, DB, NS]
        )
        in_maps1.append({"xb8": xb8, "we8": we8, "w28": w28})

    res1 = run_bass_kernel_spmd(l1, in_maps1, core_ids=list(range(NCORES))).results
    h0T = [r["h0T"] for r in res1]                            # [128, EB, NS] bf16
    lp = [np.asarray(r["lp"], dtype=np.float32) / W2_SCALE for r in res1]  # [4, NS]

    # ---- host glue: argmax -> exact critical instance -> v ----
    k_l = be @ wi + bi                                        # [2]
    c_p = be @ wb[E:]                                         # [2]
    scale = np.float32(E) ** 0.5
    wb_top = wb[:E]

    vcols = [None] * NCORES
    crit = [None] * B
    for b in range(B):
        c0, c1 = 2 * b, 2 * b + 1
        logits = np.concatenate([lp[c0][0:2], lp[c1][0:2]], axis=1)  # [2, N]
        sc = (logits + k_l[:, None]).max(axis=0)              # [N]
        iw = int(sc.argmax())
        cr = (x[b, iw].astype(np.float64) @ we + be)          # exact f64 crit
        crit[b] = cr
        q = cr @ wq + bq
        v = (wk.astype(np.float64) @ q) / scale               # [E]
        vc = np.ascontiguousarray(
            v.astype(NPBF16).reshape(EB, 128).T               # [128, EB]
        )
        vcols[c0] = vc
        vcols[c1] = vc

    in_maps2 = [{"h0T": h0T[c], "vcol": vcols[c]} for c in range(NCORES)]
    res2 = run_bass_kernel_spmd(l2, in_maps2, core_ids=list(range(NCORES))).results

    # ---- host: softmax over full bag + weighted p-sum + head ----
    out = np.zeros((B, C), dtype=np.float32)
    for b in range(B):
        c0, c1 = 2 * b, 2 * b + 1
        s = np.concatenate(
            [np.asarray(res2[c0]["s"][0]), np.asarray(res2[c1]["s"][0])]
        ).astype(np.float64)                                  # [N]
        p0 = np.concatenate([lp[c0][2:4], lp[c1][2:4]], axis=1).T  # [N, 2]
        w = np.exp(s - s.max())
        S = w.sum()
        U0 = w @ p0.astype(np.float64)                        # [2]
        attn = U0 / S + c_p
        out[b] = (crit[b] @ wb_top + attn + bb).astype(np.float32)
    return out


# revision 4
# speedup vs baseline: 2.5882x; 2.5882x over previous
"""DSMIL pooling kernel for 8 Trainium2 NeuronCores.

Sharding: B=4 bags x 2-way sequence split of N=16384 -> 8 shards of
[8192, 1024] (fp8), one per core.

Key identity: the DSMIL output is
    out = crit @ wb_top + (sum_n w_n * p_n) / (sum_n w_n) + consts
with p_n = h_n @ wb_bot a rank-2 projection, w_n = exp(s_n - max),
s_n = v . h_n, and v = wk @ (crit @ wq + bq) / sqrt(E). Since
h_n = we^T x_n (+be, which only shifts s_n by a constant that cancels
in softmax), the full [N,512] embedding h never needs to exist:

  launch 1: lp = x8 @ [we@wi | we@wb_bot]  (rank-4 fp8 DoubleRow sweep)
            -> instance logits (argmax) and p_n, shipped to host.
  host:     iw = argmax instance score; crit = x[iw] @ we + be in f64
            (exact -- fp8 crit fails the tolerance, exact crit passes
            with 40x margin); u = we @ wk @ (crit@wq+bq) / sqrt(E).
  launch 2: s = x8 @ u8  (rank-1 fp8 DoubleRow sweep) -> host.
  host:     softmax + weighted p-sum + head (tiny, f64).

Both launches are pure HBM-bandwidth sweeps of the 8MB fp8 shard.
fp8 weights are pre-scaled on host (W2*64, u*256) to dodge the e4m3
denormal range; host divides the results back.
"""

import numpy as np
import ml_dtypes

import concourse.mybir as mybir
import concourse.tile as tile
from concourse import bacc
from concourse.bass import ts
from concourse.bass_utils import run_bass_kernel_spmd

# ---- tile-tail drain workaround (this walrus build rejects >1 sync-wait
# per instruction on the kernel-tail Drain) ----
from concourse.vector_clock import ScopedClock

_MAX_WAITS = 1


def _patched_drain_and_barrier(self, tick_clock, wait_clock):
    probe = self.nc.sync.nop(nofuse=True, hint="tile_drain_waits")
    wait_clock.add_sem_waits(probe.ins, ScopedClock({None: tick_clock.global_clock}))
    si = probe.ins.sync_info
    waits = list(si.on_wait) if si is not None and si.on_wait else []
    if len(waits) > _MAX_WAITS:
        si.on_wait = waits[:_MAX_WAITS]
        rest = waits[_MAX_WAITS:]
        for k in range(0, len(rest), _MAX_WAITS):
            extra = self.nc.sync.nop(nofuse=True, hint="tile_drain_waits")
            esi = extra.ins.sync_info
            if esi is None:
                extra.ins.sync_info = mybir.SyncInfo(
                    on_wait=rest[k : k + _MAX_WAITS], on_update=[]
                )
            else:
                esi.on_wait = rest[k : k + _MAX_WAITS]
    self.nc.sync.drain()
    self.nc.all_engine_barrier()
    popped = self.nc._tile_sem_poison_stack.pop()
    assert popped is self._sem_poison
    self.nc.clear_and_free_semaphores(list(self.sems.allocated().values()))
    self.nc.all_engine_barrier()


tile.TileContext._drain_and_barrier = _patched_drain_and_barrier

F32 = mybir.dt.float32
F8 = mybir.dt.float8e4
NPF8 = ml_dtypes.float8_e4m3

B, N, D, E, C = 4, 16384, 1024, 512, 2
NCORES = 8
NS = N // 2          # per-core sequence shard
DB = D // 128        # 8
NTD = 2048           # DMA tile width (n)
NTI = 512            # inner compute tile width (psum free dim)
N_DMAT = NS // NTD   # 4
N_INNER = NTD // NTI  # 4

W2_SCALE = 64.0      # fp8 pre-scale for [we@wi | we@wb_bot]
U_SCALE = 256.0      # fp8 pre-scale for u

_cache = {}


def _build_launch1(use_dr):
    nc = bacc.Bacc(None, target_bir_lowering=False)
    x_d = nc.dram_tensor("xb8", [128, DB, NS], F8, kind="ExternalInput")
    w2_d = nc.dram_tensor("w28", [128, DB, 4], F8, kind="ExternalInput")
    lp_d = nc.dram_tensor("lp", [4, NS], F32, kind="ExternalOutput")

    with tile.TileContext(nc) as tc:
        with (
            tc.tile_pool(name="wpool", bufs=1) as wp,
            tc.tile_pool(name="xpool", bufs=2) as xp,
            tc.tile_pool(name="lpool", bufs=1) as lpp,
            tc.tile_pool(name="psum", bufs=1, space="PSUM") as pp,
        ):
            w2_sb = wp.tile([128, DB, 4], F8)
            nc.sync.dma_start(w2_sb[:], w2_d[:])
            lp_sb = lpp.tile([4, NS], F32)

            for td in range(N_DMAT):
                x_t = xp.tile([128, DB, NTD], F8, tag="xt", name="x_t")
                nc.sync.dma_start(x_t[:], x_d[:, :, ts(td, NTD)])
                for i in range(N_INNER):
                    nt = td * N_INNER + i
                    plp = pp.tile([4, NTI], F32, tag=f"plp{nt % 2}", name="plp")
                    if use_dr:
                        for dbp in range(DB // 2):
                            nc.tensor.matmul(
                                plp[:],
                                lhsT=w2_sb[:, 2 * dbp : 2 * dbp + 2, :],
                                rhs=x_t[:, 2 * dbp : 2 * dbp + 2, ts(i, NTI)],
                                start=(dbp == 0),
                                stop=(dbp == DB // 2 - 1),
                                perf_mode=mybir.MatmulPerfMode.DoubleRow,
                            )
                    else:
                        for db in range(DB):
                            nc.tensor.matmul(
                                plp[:],
                                lhsT=w2_sb[:, db, :],
                                rhs=x_t[:, db, ts(i, NTI)],
                                start=(db == 0),
                                stop=(db == DB - 1),
                            )
                    if nt % 2 == 0:
                        nc.vector.tensor_copy(lp_sb[:, ts(nt, NTI)], plp[:])
                    else:
                        nc.scalar.activation(
                            lp_sb[:, ts(nt, NTI)], plp[:],
                            mybir.ActivationFunctionType.Copy,
                        )
            nc.sync.dma_start(lp_d[:], lp_sb[:])
    nc.compile()
    return nc


def _build_launch2(use_dr):
    nc = bacc.Bacc(None, target_bir_lowering=False)
    x_d = nc.dram_tensor("xb8", [128, DB, NS], F8, kind="ExternalInput")
    u_d = nc.dram_tensor("u8", [128, DB], F8, kind="ExternalInput")
    s_d = nc.dram_tensor("s", [1, NS], F32, kind="ExternalOutput")

    with tile.TileContext(nc) as tc:
        with (
            tc.tile_pool(name="wpool", bufs=1) as wp,
            tc.tile_pool(name="xpool", bufs=2) as xp,
            tc.tile_pool(name="spool", bufs=1) as sp,
            tc.tile_pool(name="psum", bufs=1, space="PSUM") as pp,
        ):
            u_sb = wp.tile([128, DB], F8)
            nc.sync.dma_start(u_sb[:], u_d[:])
            s_sb = sp.tile([1, NS], F32)

            for td in range(N_DMAT):
                x_t = xp.tile([128, DB, NTD], F8, tag="xt", name="x_t")
                nc.sync.dma_start(x_t[:], x_d[:, :, ts(td, NTD)])
                for i in range(N_INNER):
                    nt = td * N_INNER + i
                    ps = pp.tile([1, NTI], F32, tag=f"ps{nt % 2}", name="ps")
                    if use_dr:
                        for dbp in range(DB // 2):
                            nc.tensor.matmul(
                                ps[:],
                                lhsT=u_sb[:, 2 * dbp : 2 * dbp + 2],
                                rhs=x_t[:, 2 * dbp : 2 * dbp + 2, ts(i, NTI)],
                                start=(dbp == 0),
                                stop=(dbp == DB // 2 - 1),
                                perf_mode=mybir.MatmulPerfMode.DoubleRow,
                            )
                    else:
                        for db in range(DB):
                            nc.tensor.matmul(
                                ps[:],
                                lhsT=u_sb[:, db : db + 1],
                                rhs=x_t[:, db, ts(i, NTI)],
                                start=(db == 0),
                                stop=(db == DB - 1),
                            )
                    if nt % 2 == 0:
                        nc.vector.tensor_copy(s_sb[0:1, ts(nt, NTI)], ps[:])
                    else:
                        nc.scalar.activation(
                            s_sb[0:1, ts(nt, NTI)], ps[:],
                            mybir.ActivationFunctionType.Copy,
                        )
            nc.sync.dma_start(s_d[:], s_sb[:])
    nc.compile()
    return nc


def _get_launches():
    if "l1" not in _cache:
        _cache["l1"] = _build_launch1(False)
        _cache["l2"] = _build_launch2(False)
    return _cache["l1"], _cache["l2"]


def kernel(x, we, be, wi, bi, wq, bq, wk, bk, wb, bb):
    x = np.asarray(x, dtype=np.float32)
    we = np.asarray(we, dtype=np.float32)
    be = np.asarray(be, dtype=np.float32)
    wi = np.asarray(wi, dtype=np.float32)
    bi = np.asarray(bi, dtype=np.float32)
    wq = np.asarray(wq, dtype=np.float32)
    bq = np.asarray(bq, dtype=np.float32)
    wk = np.asarray(wk, dtype=np.float32)
    bk = np.asarray(bk, dtype=np.float32)
    wb = np.asarray(wb, dtype=np.float32)
    bb = np.asarray(bb, dtype=np.float32)

    l1, l2 = _get_launches()

    W2 = np.concatenate([we @ wi, we @ wb[E:]], axis=1)  # [D, 4]
    w28 = np.ascontiguousarray(
        (W2 * W2_SCALE).astype(NPF8).reshape(DB, 128, 4).transpose(1, 0, 2)
    )

    xb8s = []
    in_maps1 = []
    for c in range(NCORES):
        b, half = divmod(c, 2)
        xs = x[b, half * NS : (half + 1) * NS, :]            # [NS, D] f32
        xb8 = np.ascontiguousarray(
            xs.astype(NPF8).reshape(NS, DB, 128).transpose(2, 1, 0)
        )                                                     # [128, DB, NS]
        xb8s.append(xb8)
        in_maps1.append({"xb8": xb8, "w28": w28})

    res1 = run_bass_kernel_spmd(l1, in_maps1, core_ids=list(range(NCORES))).results
    lp = [np.asarray(r["lp"], dtype=np.float32) / W2_SCALE for r in res1]  # [4, NS]

    # ---- host glue: argmax -> exact critical instance -> u ----
    k_l = be @ wi + bi                                        # [2]
    c_p = be @ wb[E:]                                         # [2]
    scale = np.float32(E) ** 0.5
    wef = we.astype(np.float64)
    wkf = wk.astype(np.float64)

    u8s = [None] * NCORES
    crit = [None] * B
    for b in range(B):
        c0, c1 = 2 * b, 2 * b + 1
        logits = np.concatenate([lp[c0][0:2], lp[c1][0:2]], axis=1)  # [2, N]
        sc = (logits + k_l[:, None]).max(axis=0)              # [N]
        iw = int(sc.argmax())
        cr = x[b, iw].astype(np.float64) @ wef + be           # exact f64 crit
        crit[b] = cr
        q = cr @ wq + bq
        v = (wkf @ q) / scale                                 # [E]
        u = wef @ v                                           # [D]
        u8 = np.ascontiguousarray(
            (u * U_SCALE).astype(NPF8).reshape(DB, 128).T     # [128, DB]
        )
        u8s[c0] = u8
        u8s[c1] = u8

    in_maps2 = [{"xb8": xb8s[c], "u8": u8s[c]} for c in range(NCORES)]
    res2 = run_bass_kernel_spmd(l2, in_maps2, core_ids=list(range(NCORES))).results

    # ---- host: softmax over full bag + weighted p-sum + head ----
    out = np.zeros((B, C), dtype=np.float32)
    for b in range(B):
        c0, c1 = 2 * b, 2 * b + 1
        s = np.concatenate(
            [np.asarray(res2[c0]["s"][0]), np.asarray(res2[c1]["s"][0])]
        ).astype(np.float64) / U_SCALE                        # [N]
        p0 = np.concatenate([lp[c0][2:4], lp[c1][2:4]], axis=1).T  # [N, 2]
        w = np.exp(s - s.max())
        S = w.sum()
        U0 = w @ p0.astype(np.float64)                        # [2]
        attn = U0 / S + c_p
        out[b] = (crit[b] @ wb[:E] + attn + bb).astype(np.float32)
    return out
